# revision 16
# baseline (speedup 1.0000x reference)
"""Trainium2 Bass kernel for a 2-layer GRU time-series binary classifier.

Model (torch GRU semantics, batch_first):
  seq1, _ = GRU(F=2048 -> H1=128)(x)        x: [64, 512, 2048]
  _,  h2 = GRU(H1 -> H2=64)(seq1)
  out = h2 @ fc_w.T + fc_b                  -> [64, 1]

Strategy: data-parallel over batch across 8 cores (8 sequences each).
The kernel is serial-chain bound (512 strictly sequential GRU steps), so
the design minimizes per-step chain latency:

  mm(r,z,n) -> sigmoid(r,z) -> scanA -> tanh -> scanB        (5 links)

using tensor_tensor_scan as a fused two-tensor multiply-add:
  scanA pairs: state=hnb; state=r*state+xn  => t2 = r*(hn+b_hhn)+xn
  scanB pairs: state=n;   state=omz*state+zh => h' = (1-z)*n + z*h
omz = sigmoid(-z_pre) comes from a second ACT op (scale=-1) and
zh = z*h runs on GPSIMD, both off the critical chain.  Both layers'
r/z/n pre-activations live in adjacent PSUM banks (L1 bank | L2 bank)
so every elementwise op covers L1 and L2 in ONE instruction via a
cross-bank access pattern; L2's matmuls use zero-padded stationaries so
its unused partitions 64-127 hold exact zeros.  All biases are folded
into PSUM by ones-matmul prefill, and per-step r/z recurrent matmuls
accumulate onto the chunked input-projection GEMM outputs in place.
"""

import numpy as np
import ml_dtypes

from concourse import bacc, tile, mybir
from concourse.bass_utils import run_bass_kernel_spmd

BF16 = ml_dtypes.bfloat16
N_CORES = 8
B, T, F = 64, 512, 2048
H1, H2 = 128, 64
B_LOC = B // N_CORES          # 8 sequences per core
CHUNK = 16                    # timesteps per GEMM chunk
NCH = T // CHUNK              # 32 chunks
LAG = 2 * CHUNK               # L2 runs 2 chunks behind L1
KT = F // 128                 # 16 K-tiles for GEMM1
NW = CHUNK * B_LOC            # 128 moving columns per chunk GEMM
NROUND = T + LAG              # 544 rounds
AF = mybir.ActivationFunctionType
ALU = mybir.AluOpType
DT_BF = mybir.dt.bfloat16
DT_F32 = mybir.dt.float32
SBUFS = 4                     # rotation depth for per-step tiles


def build_nc():
    nc = bacc.Bacc(None, target_bir_lowering=False)

    xT = nc.declare_dram_parameter("xT", [F, T, B_LOC], DT_BF, isOutput=False)
    wih1T = nc.declare_dram_parameter("wih1T", [F, 3 * H1], DT_BF, isOutput=False)
    whh1T = nc.declare_dram_parameter("whh1T", [H1, 3 * 128], DT_BF, isOutput=False)
    wih2T = nc.declare_dram_parameter("wih2T", [H1, 3 * 128], DT_BF, isOutput=False)
    whh2T = nc.declare_dram_parameter("whh2T", [H2, 3 * 128], DT_BF, isOutput=False)
    brow1 = nc.declare_dram_parameter("brow1", [1, 3 * 128], DT_BF, isOutput=False)
    brow2 = nc.declare_dram_parameter("brow2", [1, 3 * 128], DT_BF, isOutput=False)
    bn1row = nc.declare_dram_parameter("bn1row", [1, 128], DT_BF, isOutput=False)
    bn2row = nc.declare_dram_parameter("bn2row", [1, 128], DT_BF, isOutput=False)
    fcwT = nc.declare_dram_parameter("fcwT", [H2, 1], DT_BF, isOutput=False)
    fcb = nc.declare_dram_parameter("fcb", [B_LOC, 1], DT_F32, isOutput=False)
    out = nc.declare_dram_parameter("out", [B_LOC, 1], DT_F32, isOutput=True)

    with tile.TileContext(nc) as tc:
        with (
            tc.tile_pool(name="const", bufs=1) as cpool,
            tc.tile_pool(name="xchunk", bufs=3) as xpool,
            tc.tile_pool(name="psum", bufs=3, space="PSUM") as ppool,
        ):
            # ---- persistent tiles -------------------------------------
            w1 = cpool.tile([128, KT, 3 * H1], DT_BF)      # GEMM1 stationaries
            wh1 = cpool.tile([H1, 3 * 128], DT_BF)
            w2 = cpool.tile([H1, 3 * 128], DT_BF)          # zero-padded
            wh2 = cpool.tile([H2, 3 * 128], DT_BF)         # zero-padded
            br1 = cpool.tile([1, 3 * 128], DT_BF)
            br2 = cpool.tile([1, 3 * 128], DT_BF)
            bn1 = cpool.tile([1, 128], DT_BF)
            bn2 = cpool.tile([1, 128], DT_BF)
            ones = cpool.tile([1, NW], DT_BF)
            fw = cpool.tile([H2, 1], DT_BF)
            fb = cpool.tile([B_LOC, 1], DT_F32)
            res = cpool.tile([B_LOC, 1], DT_F32)
            # h history: per round-slot 32 cols = (layer:2) x (b:8) x
            # (junk|h:2).  h_t real values sit at odd cols.
            hh = cpool.tile([128, (NROUND + 2) * 32], DT_BF)

            nc.sync.dma_start(out=w1[:], in_=wih1T.rearrange("(kt p) g -> p kt g", p=128))
            nc.sync.dma_start(out=wh1[:], in_=whh1T[:])
            nc.sync.dma_start(out=w2[:], in_=wih2T[:])
            nc.sync.dma_start(out=wh2[:], in_=whh2T[:])
            nc.sync.dma_start(out=br1[:], in_=brow1[:])
            nc.sync.dma_start(out=br2[:], in_=brow2[:])
            nc.sync.dma_start(out=bn1[:], in_=bn1row[:])
            nc.sync.dma_start(out=bn2[:], in_=bn2row[:])
            nc.sync.dma_start(out=fw[:], in_=fcwT[:])
            nc.sync.dma_start(out=fb[:], in_=fcb[:])
            nc.vector.memset(ones[:], 1.0)

            hh_r = hh.rearrange("p (t l b w) -> p t l b w", l=2, b=B_LOC, w=2)
            # h_0 = 0 for L1 (slot 0) and L2 (slot LAG)
            nc.vector.memset(hh[:, 0:32], 0.0)
            nc.vector.memset(hh[:, LAG * 32:(LAG + 1) * 32], 0.0)

            # Persistent per-step tiles: the serial h-chain already orders
            # round t's readers before round t+1's writers, so one instance
            # suffices.  dz/d0B even columns are the scans' zero slots --
            # memset once; only odds are ever rewritten.
            dz = cpool.tile([128, 64], DT_F32)
            d0B = cpool.tile([128, 32], DT_F32)
            d1B = cpool.tile([128, 32], DT_F32)
            t2s = cpool.tile([128, 32], DT_F32)
            nc.vector.memset(dz[:], 0.0)
            nc.vector.memset(d0B[:], 0.0)
            dzr = dz.rearrange("p (g2 l b w) -> p l g2 b w", g2=2, l=2, w=2)
            d0Br = d0B.rearrange("p (l b w) -> p l b w", l=2, w=2)
            d1Br = d1B.rearrange("p (l b w) -> p l b w", l=2, w=2)
            t2sr = t2s.rearrange("p (l b w) -> p l b w", l=2, w=2)

            # ---- chunk-state ------------------------------------------
            xtiles = {}
            pts = {}     # chunk-round r -> psum tile [128, 1024] (L1|L2 banks)

            def alloc_pt(r):
                pts[r] = ppool.tile([128, 1024], DT_F32, tag="pt", name="pt")
                # All matmuls into pt use start=False and accumulate onto
                # explicit zeros (the sim's pending-zero model can't track
                # interleaved stride-2 dsts next to start=True banks).
                nc.vector.memset(pts[r][:], 0.0)

            def dma_xchunk(c):
                xt = xpool.tile([128, KT, NW], DT_BF, tag="xc", name="xc")
                nc.sync.dma_start(
                    out=xt[:],
                    in_=xT[:, c * CHUNK:(c + 1) * CHUNK, :].rearrange(
                        "(kt p) t b -> p kt (t b)", p=128),
                )
                xtiles[c] = xt

            # PSUM layout per chunk-round (pt = [128, 1024] = 2 banks):
            #   n  bank: L1 (hnb|xn) pairs [0:256] (col 16t+2b+w),
            #            L2 pairs [256:512]
            #   rz bank: L1r [512:640] (col 8t+b), L1z [640:768],
            #            L2r [768:896], L2z [896:1024]
            # GEMM dsts are contiguous or stride-2 2-D slices; the scans
            # read contiguous 16-col windows (scanA split per layer, both on
            # DVE: GPSIMD cannot access PSUM).
            def _rzv(c):
                return pts[c][:, 512:1024].rearrange(
                    "p (l g tb) -> p l g tb", l=2, g=2)

            def _nv(c, l):
                return pts[c][:, 256 * l:256 * l + 256].rearrange(
                    "p (j w) -> p j w", w=2)

            def gemm1_thunks(c):
                """Layer-1 input projection of chunk c.  Its first matmul
                into each bank carries start=True (clears has_written for
                the whole bank)."""
                xt = xtiles[c]
                pt = pts[c]
                nL1 = _nv(c, 0)
                thunks = []
                for g, dst in ((0, pt[:, 512:640]), (1, pt[:, 640:768]),
                               (2, nL1[:, :, 1])):
                    def mk(kt, g=g, dst=dst):
                        def f():
                            nc.tensor.matmul(
                                dst, w1[:, kt, g * 128:(g + 1) * 128], xt[:, kt],
                                start=False, stop=False,
                                skip_group_check=True)
                        return f
                    for kt in range(KT):
                        thunks.append(mk(kt))

                    def fbias(g=g, dst=dst):
                        nc.tensor.matmul(
                            dst, br1[:, g * 128:(g + 1) * 128], ones[:],
                            start=False, stop=(g == 2), skip_group_check=True)
                    thunks.append(fbias)

                def fpre():
                    nc.tensor.matmul(
                        nL1[:, :, 0], bn1[:], ones[:],
                        start=False, stop=False, skip_group_check=True)
                thunks.append(fpre)
                return thunks

            def gemm2_thunks(j, first):
                """Layer-2 input projection of its chunk j (consumes h1
                history rounds 16j..16j+15) into round-chunk (j+2).
                `first`: no gemm1 shares this round, so carry the bank
                clears here."""
                pt = pts[j + 2]
                nL2 = _nv(j + 2, 1)
                mv = hh_r[:, CHUNK * j + 1:CHUNK * j + 1 + CHUNK, 0, :, 1]
                thunks = []
                for g, dst in ((0, pt[:, 768:896]), (1, pt[:, 896:1024]),
                               (2, nL2[:, :, 1])):
                    def fmm(g=g, dst=dst):
                        nc.tensor.matmul(
                            dst, w2[:, g * 128:(g + 1) * 128], mv,
                            start=False, stop=False,
                            skip_group_check=True)
                    thunks.append(fmm)

                    def fbias(g=g, dst=dst):
                        nc.tensor.matmul(
                            dst, br2[:, g * 128:(g + 1) * 128], ones[:],
                            start=False, stop=(g == 2), skip_group_check=True)
                    thunks.append(fbias)

                def fpre():
                    nc.tensor.matmul(
                        nL2[:, :, 0], bn2[:], ones[:],
                        start=False, stop=False, skip_group_check=True)
                thunks.append(fpre)
                return thunks

            def round_step(s):
                """One merged GRU step for both layers at round s.
                L1 computes its step s; L2 computes its step s-LAG."""
                t = s % CHUNK
                c = s // CHUNK
                pt = pts[c]
                rzv = _rzv(c)
                lo, hi = (0, 2)
                if s < LAG:
                    lo, hi = 0, 1          # L1 only
                elif s >= T:
                    lo, hi = 1, 2          # L2 only

                # --- recurrent matmuls (accumulate onto GEMM psum) ----
                # r,z gates first (they gate the sigmoid); n gates last.
                hprev = hh_r[:, s, :, :, 1]
                mms = []
                if lo == 0:
                    mv1 = hprev[:, 0, :]
                    nL1 = _nv(c, 0)
                    mms += [(wh1, mv1, 0, pt[:, 512 + 8 * t:512 + 8 * t + 8]),
                            (wh1, mv1, 1, pt[:, 640 + 8 * t:640 + 8 * t + 8])]
                if hi == 2:
                    mv2 = hprev[0:H2, 1, :]
                    nL2 = _nv(c, 1)
                    mms += [(wh2, mv2, 0, pt[:, 768 + 8 * t:768 + 8 * t + 8]),
                            (wh2, mv2, 1, pt[:, 896 + 8 * t:896 + 8 * t + 8])]
                if lo == 0:
                    mms.append((wh1, mv1, 2, nL1[:, 8 * t:8 * t + 8, 0]))
                if hi == 2:
                    mms.append((wh2, mv2, 2, nL2[:, 8 * t:8 * t + 8, 0]))
                for wh, mv, g, dst in mms:
                    nc.tensor.matmul(
                        dst, wh[:, g * 128:(g + 1) * 128], mv,
                        start=False, stop=True, skip_group_check=True)

                # --- sigmoid r,z -> dz (r pairs cols 0:32, z 32:64) ----
                nc.scalar.activation(
                    dzr[:, lo:hi, :, :, 1], rzv[:, lo:hi, :, 8 * t:8 * t + 8],
                    AF.Sigmoid)

                # --- omz = sigmoid(-z_pre) -> d0B odds ----------------
                # (emitted before the scans: the framework's WAR wait uses
                # the DVE sem value at emission time, and omz must not wait
                # for this round's scans)
                nc.scalar.activation(
                    d0Br[:, lo:hi, :, 1], rzv[:, lo:hi, 1, 8 * t:8 * t + 8],
                    AF.Sigmoid, scale=-1.0)

                # --- scanA: t2 = r*(hn + b_hhn) + xn (per layer, DVE)
                if lo == 0:
                    nc.vector.tensor_tensor_scan(
                        out=t2s[:, 0:16], data0=dz[:, 0:16],
                        data1=pt[:, 16 * t:16 * t + 16],
                        initial=0.0, op0=ALU.mult, op1=ALU.add)
                if hi == 2:
                    nc.vector.tensor_tensor_scan(
                        out=t2s[:, 16:32], data0=dz[:, 16:32],
                        data1=pt[:, 256 + 16 * t:256 + 16 * t + 16],
                        initial=0.0, op0=ALU.mult, op1=ALU.add)

                # --- zh = z*h on gpsimd -> d1B odds -------------------
                nc.gpsimd.tensor_tensor(
                    out=d1Br[:, lo:hi, :, 1], in0=dzr[:, lo:hi, 1, :, 1],
                    in1=hprev[:, lo:hi, :], op=ALU.mult)

                # --- tanh -> d1B evens --------------------------------
                nc.scalar.activation(
                    d1Br[:, lo:hi, :, 0], t2sr[:, lo:hi, :, 1], AF.Tanh)

                # --- scanB: h' = omz*n + zh ---------------------------
                nc.vector.tensor_tensor_scan(
                    out=hh[:, 32 * (s + 1) + 16 * lo:32 * (s + 1) + 16 * hi],
                    data0=d0B[:, 16 * lo:16 * hi], data1=d1B[:, 16 * lo:16 * hi],
                    initial=0.0, op0=ALU.mult, op1=ALU.add)

            # ---- prologue --------------------------------------------
            dma_xchunk(0)
            dma_xchunk(1)
            alloc_pt(0)
            for f in gemm1_thunks(0):
                f()

            # ---- main loop -------------------------------------------
            thunks = []
            for s in range(NROUND):
                if s % CHUNK == 0:
                    k = s // CHUNK
                    if k + 1 <= NCH + 1:
                        alloc_pt(k + 1)
                    if k + 1 < NCH:
                        thunks += gemm1_thunks(k + 1)
                    if 0 <= k - 1 < NCH:
                        thunks += gemm2_thunks(k - 1, first=(k + 1 >= NCH))
                    if k + 2 < NCH:
                        dma_xchunk(k + 2)
                round_step(s)
                for _ in range(4):
                    if thunks:
                        thunks.pop(0)()
            while thunks:
                thunks.pop(0)()

            # ---- fc head ---------------------------------------------
            h2fin = hh_r[0:H2, NROUND, 1, :, 1]            # [64, 8] bf16
            fcp = ppool.tile([B_LOC, 1], DT_F32, tag="fc", name="fcp", bufs=1)
            nc.tensor.matmul(fcp[:], h2fin, fw[:], start=True, stop=True,
                             skip_group_check=True)
            nc.scalar.activation(res[:], fcp[:], AF.Identity, bias=fb[:])
            nc.sync.dma_start(out=out[:], in_=res[:])

    nc.compile()
    return nc


_NC_CACHE = {}


def _get_nc():
    if "nc" not in _NC_CACHE:
        _NC_CACHE["nc"] = build_nc()
    return _NC_CACHE["nc"]


def _pad_gates(m, hin):
    """[3*H2, hin] torch-layout weight -> [hin, 3*128] bf16 stationary with
    zero padding in output channels 64..127 of each gate."""
    out = np.zeros((hin, 3 * 128), dtype=np.float32)
    for g in range(3):
        out[:, g * 128:g * 128 + H2] = m[g * H2:(g + 1) * H2, :].T
    return out.astype(BF16)


def _prep_maps(x, w_ih1, w_hh1, b_ih1, b_hh1, w_ih2, w_hh2, b_ih2, b_hh2,
               fc_w, fc_b):
    f32 = np.float32
    brow1 = np.concatenate([
        (b_ih1[:H1] + b_hh1[:H1]),
        (b_ih1[H1:2 * H1] + b_hh1[H1:2 * H1]),
        b_ih1[2 * H1:],                         # n gate: b_ih only
    ]).reshape(1, 3 * 128)
    brow2 = np.zeros((1, 3 * 128), dtype=f32)
    brow2[0, 0:H2] = b_ih2[:H2] + b_hh2[:H2]
    brow2[0, 128:128 + H2] = b_ih2[H2:2 * H2] + b_hh2[H2:2 * H2]
    brow2[0, 256:256 + H2] = b_ih2[2 * H2:]
    bn2row = np.zeros((1, 128), dtype=f32)
    bn2row[0, :H2] = b_hh2[2 * H2:]
    shared = {
        "wih1T": np.ascontiguousarray(w_ih1.T).astype(BF16),
        "whh1T": np.ascontiguousarray(
            np.concatenate([w_hh1[g * H1:(g + 1) * H1, :].T for g in range(3)],
                           axis=1)).astype(BF16),
        "wih2T": _pad_gates(w_ih2, H1),
        "whh2T": _pad_gates(w_hh2, H2),
        "brow1": brow1.astype(BF16),
        "brow2": brow2.astype(BF16),
        "bn1row": np.ascontiguousarray(
            b_hh1[2 * H1:].reshape(1, 128)).astype(BF16),
        "bn2row": bn2row.astype(BF16),
        "fcwT": np.ascontiguousarray(fc_w.reshape(1, H2).T).astype(BF16),
        "fcb": np.full((B_LOC, 1), float(fc_b.reshape(-1)[0]), dtype=f32),
    }
    maps = []
    for c in range(N_CORES):
        xc = x[c * B_LOC:(c + 1) * B_LOC]          # [B_LOC, T, F]
        xTc = np.ascontiguousarray(xc.transpose(2, 1, 0)).astype(BF16)
        maps.append({"xT": xTc, **shared})
    return maps


def run(inputs, trace=False):
    nc = _get_nc()
    maps = _prep_maps(**inputs)
    res = run_bass_kernel_spmd(nc, maps, list(range(N_CORES)), trace=trace)
    outs = [np.asarray(res.results[i]["out"], np.float32) for i in range(N_CORES)]
    full = np.concatenate(outs, axis=0)            # [64, 1]
    return full, res.exec_time_ns


def kernel(**inputs):
    inputs = {k: np.asarray(v, np.float32) for k, v in inputs.items()}
    out, _ = run(inputs, trace=False)
    return out


# revision 18
# speedup vs baseline: 1.1067x; 1.1067x over previous
"""Trainium2 Bass kernel for a 2-layer GRU time-series binary classifier.

Model (torch GRU semantics, batch_first):
  seq1, _ = GRU(F=2048 -> H1=128)(x)        x: [64, 512, 2048]
  _,  h2 = GRU(H1 -> H2=64)(seq1)
  out = h2 @ fc_w.T + fc_b                  -> [64, 1]

Strategy: data-parallel over batch across 8 cores (8 sequences each).
Per core, layer-1's input projection runs as a chunked bf16 GEMM whose
gate outputs stay in PSUM; the recurrent h @ W_hh.T matmuls accumulate
into the same PSUM banks (start=False on set has_written bits), so the
r/z gates need no explicit adds.  State h lives in [H, B] layout so no
transposes appear anywhere.  Layer 2 runs one 32-step chunk behind
layer 1; its input projection consumes layer-1's h history directly.
"""

import numpy as np
import ml_dtypes

from concourse import bacc, tile, mybir
from concourse.bass_utils import run_bass_kernel_spmd

BF16 = ml_dtypes.bfloat16
N_CORES = 8
B, T, F = 64, 512, 2048
H1, H2 = 128, 64
B_LOC = B // N_CORES          # 8 sequences per core
CHUNK = 32                    # timesteps per GEMM chunk
NCH = T // CHUNK              # 16 chunks
AF = mybir.ActivationFunctionType
ALU = mybir.AluOpType
DT_BF = mybir.dt.bfloat16
DT_F32 = mybir.dt.float32


def build_nc():
    nc = bacc.Bacc(None, target_bir_lowering=False)

    xT = nc.declare_dram_parameter("xT", [F, T, B_LOC], DT_BF, isOutput=False)
    wih1T = nc.declare_dram_parameter("wih1T", [F, 3 * H1], DT_BF, isOutput=False)
    whh1T = nc.declare_dram_parameter("whh1T", [H1, 3 * H1], DT_BF, isOutput=False)
    wih2T = nc.declare_dram_parameter("wih2T", [H1, 3 * H2], DT_BF, isOutput=False)
    whh2T = nc.declare_dram_parameter("whh2T", [H2, 3 * H2], DT_BF, isOutput=False)
    brow1 = nc.declare_dram_parameter("brow1", [1, 3 * H1], DT_BF, isOutput=False)
    brow2 = nc.declare_dram_parameter("brow2", [1, 3 * H2], DT_BF, isOutput=False)
    bhn1 = nc.declare_dram_parameter("bhn1", [H1, 1], DT_F32, isOutput=False)
    bhn2 = nc.declare_dram_parameter("bhn2", [H2, 1], DT_F32, isOutput=False)
    fcwT = nc.declare_dram_parameter("fcwT", [H2, 1], DT_F32, isOutput=False)
    fcb = nc.declare_dram_parameter("fcb", [B_LOC, 1], DT_F32, isOutput=False)
    out = nc.declare_dram_parameter("out", [B_LOC, 1], DT_F32, isOutput=True)

    KT = F // 128              # 16 K-tiles for GEMM1
    NW = CHUNK * B_LOC         # 256 moving columns per chunk GEMM

    with tile.TileContext(nc) as tc:
        with (
            tc.tile_pool(name="const", bufs=1) as cpool,
            tc.tile_pool(name="xchunk", bufs=3) as xpool,
            tc.tile_pool(name="xn", bufs=2) as xnpool,
            tc.tile_pool(name="step", bufs=3) as spool,
            tc.tile_pool(name="psum", bufs=2, space="PSUM") as ppool,
        ):
            # ---- persistent tiles -------------------------------------
            w1 = cpool.tile([128, KT, 3 * H1], DT_BF)      # GEMM1 stationaries
            wh1 = cpool.tile([H1, 3 * H1], DT_BF)
            w2 = cpool.tile([H1, 3 * H2], DT_BF)
            wh2 = cpool.tile([H2, 3 * H2], DT_BF)
            br1 = cpool.tile([1, 3 * H1], DT_BF)
            br2 = cpool.tile([1, 3 * H2], DT_BF)
            bn1 = cpool.tile([H1, 1], DT_F32)
            bn2 = cpool.tile([H2, 1], DT_F32)
            fw = cpool.tile([H2, 1], DT_F32)
            fb = cpool.tile([B_LOC, 1], DT_F32)
            ones = cpool.tile([1, NW], DT_BF)
            h1h = cpool.tile([H1, (T + 1) * B_LOC], DT_BF)  # h1 history
            h2h = cpool.tile([H2, (T + 1) * B_LOC], DT_BF)
            h2fin = cpool.tile([H2, B_LOC], DT_F32)

            nc.sync.dma_start(out=w1[:], in_=wih1T.rearrange("(kt p) g -> p kt g", p=128))
            nc.sync.dma_start(out=wh1[:], in_=whh1T[:])
            nc.sync.dma_start(out=w2[:], in_=wih2T[:])
            nc.sync.dma_start(out=wh2[:], in_=whh2T[:])
            nc.sync.dma_start(out=br1[:], in_=brow1[:])
            nc.sync.dma_start(out=br2[:], in_=brow2[:])
            nc.sync.dma_start(out=bn1[:], in_=bhn1[:])
            nc.sync.dma_start(out=bn2[:], in_=bhn2[:])
            nc.sync.dma_start(out=fw[:], in_=fcwT[:])
            nc.sync.dma_start(out=fb[:], in_=fcb[:])
            nc.vector.memset(ones[:], 1.0)
            nc.vector.memset(h1h[:, 0:B_LOC], 0.0)
            nc.vector.memset(h2h[:, 0:B_LOC], 0.0)

            # ---- chunk-state carried across waves ---------------------
            xtiles = {}       # chunk -> x SBUF tile [128, KT, NW]
            rz1_ps = {}       # chunk -> psum [128, 512]: r | z
            n1_ps = {}        # chunk -> psum [128, 512]: xn gemm | step hn
            rz2_ps = {}
            n2_ps = {}
            xn1_sb = {}
            xn2_sb = {}

            def dma_xchunk(c):
                xt = xpool.tile([128, KT, NW], DT_BF, tag="xc")
                nc.sync.dma_start(
                    out=xt[:],
                    in_=xT[:, c * CHUNK:(c + 1) * CHUNK, :].rearrange(
                        "(kt p) t b -> p kt (t b)", p=128),
                )
                xtiles[c] = xt

            def gemm1_closures(c):
                """Emission thunks for layer-1 input projection of chunk c."""
                rz = ppool.tile([128, 512], DT_F32, tag="l1rz")
                np_ = ppool.tile([128, 512], DT_F32, tag="l1n")
                rz1_ps[c], n1_ps[c] = rz, np_
                xt = xtiles[c]
                thunks = []
                for g, (dst, lo) in enumerate(
                    [(rz, 0), (rz, 256), (np_, 0)]):  # r, z, n
                    # start=True clears has_written for the WHOLE bank, so
                    # only the first matmul touching each bank may set it
                    # (z rides on r's clear; per-element bits handle the rest).
                    def mk(kt, g=g, dst=dst, lo=lo):
                        def f():
                            nc.tensor.matmul(
                                dst[:, lo:lo + NW],
                                w1[:, kt, g * 128:(g + 1) * 128],
                                xt[:, kt],
                                start=(kt == 0 and lo == 0), stop=False,
                                skip_group_check=True)
                        return f
                    for kt in range(KT):
                        thunks.append(mk(kt))

                    def fbias(g=g, dst=dst, lo=lo):
                        nc.tensor.matmul(
                            dst[:, lo:lo + NW],
                            br1[:, g * 128:(g + 1) * 128],
                            ones[:],
                            start=False, stop=True,
                            skip_group_check=True)
                    thunks.append(fbias)

                xs = xnpool.tile([128, NW], DT_F32, tag="xn1")
                xn1_sb[c] = xs

                def fdrain():
                    nc.scalar.copy(xs[:], np_[:, 0:NW])
                thunks.append(fdrain)
                return thunks

            def gemm2_closures(c):
                """Layer-2 input projection of chunk c (reads h1 history)."""
                rz = ppool.tile([H2, 512], DT_F32, tag="l2rz")
                np_ = ppool.tile([H2, 512], DT_F32, tag="l2n")
                rz2_ps[c], n2_ps[c] = rz, np_
                mv = h1h[:, (c * CHUNK + 1) * B_LOC:(c * CHUNK + 1 + CHUNK) * B_LOC]
                thunks = []
                for g, (dst, lo) in enumerate(
                    [(rz, 0), (rz, 256), (np_, 0)]):
                    def fmm(g=g, dst=dst, lo=lo):
                        nc.tensor.matmul(
                            dst[:, lo:lo + NW],
                            w2[:, g * H2:(g + 1) * H2],
                            mv,
                            start=(lo == 0), stop=False,
                            skip_group_check=True)
                    thunks.append(fmm)

                    def fbias(g=g, dst=dst, lo=lo):
                        nc.tensor.matmul(
                            dst[:, lo:lo + NW],
                            br2[:, g * H2:(g + 1) * H2],
                            ones[:],
                            start=False, stop=True,
                            skip_group_check=True)
                    thunks.append(fbias)

                xs = xnpool.tile([H2, NW], DT_F32, tag="xn2")
                xn2_sb[c] = xs

                def fdrain():
                    nc.scalar.copy(xs[:], np_[:, 0:NW])
                thunks.append(fdrain)
                return thunks

            def step_phases(layer, c, t):
                """One GRU cell update, [H, B] layout, split into phases so
                the two layers' chains can interleave per-engine instead of
                serializing in each engine's in-order queue."""
                if layer == 1:
                    H, hh, wh, bn, rzp, npp, xns = (
                        H1, h1h, wh1, bn1, rz1_ps[c], n1_ps[c], xn1_sb[c])
                else:
                    H, hh, wh, bn, rzp, npp, xns = (
                        H2, h2h, wh2, bn2, rz2_ps[c], n2_ps[c], xn2_sb[c])
                gt = c * CHUNK + t
                hp = hh[:, gt * B_LOC:(gt + 1) * B_LOC]
                co = t * B_LOC                        # column offset in chunk
                so = 256 + co                         # step region offset
                st = {}

                def ph_mm():
                    # recurrent matmuls: r,z accumulate onto GEMM psum; n fresh
                    nc.tensor.matmul(rzp[:, co:co + B_LOC], wh[:, 0:H], hp,
                                     start=False, stop=True,
                                     skip_group_check=True)
                    nc.tensor.matmul(rzp[:, 256 + co:256 + co + B_LOC],
                                     wh[:, H:2 * H], hp,
                                     start=False, stop=True,
                                     skip_group_check=True)
                    nc.tensor.matmul(npp[:, so:so + B_LOC], wh[:, 2 * H:3 * H],
                                     hp, start=True, stop=True,
                                     skip_group_check=True)

                def ph_sig():
                    rzv = rzp.rearrange("p (g x) -> p g x", g=2)[:, :, co:co + B_LOC]
                    rz_t = spool.tile([H, 2, B_LOC], DT_F32, tag=f"rz{layer}")
                    st["rz"] = rz_t
                    nc.scalar.activation(rz_t[:], rzv, AF.Sigmoid)

                def ph_tn():
                    # n = tanh(xn + r * (hn + b_hhn))
                    tn = spool.tile([H, B_LOC], DT_F32, tag=f"tn{layer}")
                    nc.vector.scalar_tensor_tensor(
                        out=tn[:], in0=npp[:, so:so + B_LOC], scalar=bn[:],
                        in1=st["rz"][:, 0], op0=ALU.add, op1=ALU.mult)
                    t2 = spool.tile([H, B_LOC], DT_F32, tag=f"t2{layer}")
                    st["t2"] = t2
                    nc.vector.tensor_tensor(out=t2[:], in0=tn[:],
                                            in1=xns[:, co:co + B_LOC],
                                            op=ALU.add)

                def ph_tanh():
                    n_t = spool.tile([H, B_LOC], DT_F32, tag=f"n{layer}")
                    st["n"] = n_t
                    nc.scalar.activation(n_t[:], st["t2"][:], AF.Tanh)

                def ph_h():
                    # h' = n + z*(h - n)
                    n_t = st["n"]
                    d = spool.tile([H, B_LOC], DT_F32, tag=f"d{layer}")
                    nc.vector.tensor_tensor(out=d[:], in0=hp, in1=n_t[:],
                                            op=ALU.subtract)
                    e = spool.tile([H, B_LOC], DT_F32, tag=f"e{layer}")
                    nc.vector.tensor_tensor(out=e[:], in0=st["rz"][:, 1],
                                            in1=d[:], op=ALU.mult)
                    if layer == 2 and gt == T - 1:
                        nc.vector.tensor_tensor(out=h2fin[:], in0=n_t[:],
                                                in1=e[:], op=ALU.add)
                    else:
                        nc.vector.tensor_tensor(
                            out=hh[:, (gt + 1) * B_LOC:(gt + 2) * B_LOC],
                            in0=n_t[:], in1=e[:], op=ALU.add)

                return [ph_mm, ph_sig, ph_tn, ph_tanh, ph_h]

            # ---- prologue --------------------------------------------
            dma_xchunk(0)
            dma_xchunk(1)
            for f in gemm1_closures(0):
                f()

            # ---- flat slot timeline ----------------------------------
            # L1 runs step s at slot s; L2 runs step s-LAG at slot s.
            # At each chunk boundary 32k we enqueue GEMM2(k-1) (h1 chunk
            # k-1 just finished) and GEMM1(k+1); thunks pop a few per
            # slot so the PE never stalls on a GEMM block and every
            # psum is fully written before its first consumer is traced.
            LAG = CHUNK + 8
            thunks = []
            for s in range(T + LAG):
                if s % CHUNK == 0:
                    k = s // CHUNK
                    if 1 <= k <= NCH:
                        thunks += gemm2_closures(k - 1)
                    if 1 <= k + 1 < NCH:
                        thunks += gemm1_closures(k + 1)
                    if k + 2 < NCH:
                        dma_xchunk(k + 2)
                p1 = step_phases(1, s // CHUNK, s % CHUNK) if s < T else []
                u = s - LAG
                p2 = step_phases(2, u // CHUNK, u % CHUNK) if 0 <= u < T else []
                for i in range(5):
                    if p1:
                        p1[i]()
                    if p2:
                        p2[i]()
                for _ in range(3):
                    if thunks:
                        thunks.pop(0)()
            while thunks:
                thunks.pop(0)()

            # ---- fc head ---------------------------------------------
            fcp = ppool.tile([B_LOC, 1], DT_F32, tag="l2rz")
            nc.tensor.matmul(fcp[:], h2fin[:], fw[:], start=True, stop=True,
                             skip_group_check=True)
            res = cpool.tile([B_LOC, 1], DT_F32)
            nc.scalar.activation(res[:], fcp[:], AF.Identity, bias=fb[:])
            nc.sync.dma_start(out=out[:], in_=res[:])

    nc.compile()
    return nc


_NC_CACHE = {}


def _get_nc():
    if "nc" not in _NC_CACHE:
        _NC_CACHE["nc"] = build_nc()
    return _NC_CACHE["nc"]


def _prep_maps(x, w_ih1, w_hh1, b_ih1, b_hh1, w_ih2, w_hh2, b_ih2, b_hh2,
               fc_w, fc_b):
    f32 = np.float32
    brow1 = np.concatenate([
        (b_ih1[:H1] + b_hh1[:H1]),
        (b_ih1[H1:2 * H1] + b_hh1[H1:2 * H1]),
        b_ih1[2 * H1:],
    ]).reshape(1, 3 * H1)
    brow2 = np.concatenate([
        (b_ih2[:H2] + b_hh2[:H2]),
        (b_ih2[H2:2 * H2] + b_hh2[H2:2 * H2]),
        b_ih2[2 * H2:],
    ]).reshape(1, 3 * H2)
    shared = {
        "wih1T": np.ascontiguousarray(w_ih1.T).astype(BF16),
        "whh1T": np.ascontiguousarray(w_hh1.T).astype(BF16),
        "wih2T": np.ascontiguousarray(w_ih2.T).astype(BF16),
        "whh2T": np.ascontiguousarray(w_hh2.T).astype(BF16),
        "brow1": brow1.astype(BF16),
        "brow2": brow2.astype(BF16),
        "bhn1": np.ascontiguousarray(b_hh1[2 * H1:].reshape(H1, 1), dtype=f32),
        "bhn2": np.ascontiguousarray(b_hh2[2 * H2:].reshape(H2, 1), dtype=f32),
        "fcwT": np.ascontiguousarray(fc_w.reshape(1, H2).T, dtype=f32),
        "fcb": np.full((B_LOC, 1), float(fc_b.reshape(-1)[0]), dtype=f32),
    }
    maps = []
    for c in range(N_CORES):
        xc = x[c * B_LOC:(c + 1) * B_LOC]          # [B_LOC, T, F]
        xTc = np.ascontiguousarray(xc.transpose(2, 1, 0)).astype(BF16)
        maps.append({"xT": xTc, **shared})
    return maps


def run(inputs, trace=False):
    nc = _get_nc()
    maps = _prep_maps(**inputs)
    res = run_bass_kernel_spmd(nc, maps, list(range(N_CORES)), trace=trace)
    outs = [np.asarray(res.results[i]["out"], np.float32) for i in range(N_CORES)]
    full = np.concatenate(outs, axis=0)            # [64, 1]
    return full, res.exec_time_ns


def kernel(**inputs):
    inputs = {k: np.asarray(v, np.float32) for k, v in inputs.items()}
    out, _ = run(inputs, trace=False)
    return out
